# revision 2
# baseline (speedup 1.0000x reference)
"""Trainium2 Bass kernel for the interval-prediction custom loss.

total = 10*mean((t - c)^2) + 0.1*mean(up-lo) + 10*mean(relu(lo-up))
        + 0.5*sum(relu(sigma*(c-p)))/N,  c = (lo+up)/2, sigma = 1-2*pv.

Data parallel over N across 8 cores. Streams per core: lo, up, t, p,
sg (= 1-2*pv as bf16, exact +-1). Per tile of FD columns:

  DVE: h = lo + up                 (TT 2x)
       c = 0.5*h                   (TS 4x)
       d = lo - c (+acc sum d)     (STT 1x, fused width reduction)
         [note d = (lo-up)/2, so width = -2*sum(d), valid relu(lo-up)=2*relu(d)]
       e = c - t                   (TT 2x)
       x = c - p                   (TT 2x)
       z = sg * x                  (TT 2x)
  ACT: sum e^2   (Square acc)
       sum relu(d)  -> valid = 2*relu(d)
       sum relu(z)  -> direction

All reductions land in [P, n_tiles] accumulators; host does the tiny
final combine in float64.
"""

import sys

if "/opt/trn_rl_repo" not in sys.path:
    sys.path.insert(0, "/opt/trn_rl_repo")

import numpy as np

N = 8388608
N_CORES = 8
P = 128
NP_PER_CORE = N // N_CORES            # 1048576
FPL = NP_PER_CORE // P                # 8192
TILE_WIDTHS = (1024, 2048, 2048, 2048, 1024)
assert sum(TILE_WIDTHS) == FPL

_NC_CACHE = {}


def _build():
    from concourse import bacc, mybir
    from concourse.tile import TileContext

    f32 = mybir.dt.float32
    bf16 = mybir.dt.bfloat16
    Alu = mybir.AluOpType
    Act = mybir.ActivationFunctionType

    n_tiles = len(TILE_WIDTHS)
    nc = bacc.Bacc(trn_type="TRN2")
    big = nc.declare_dram_parameter("big", [P, 5 * FPL], bf16, isOutput=False)
    out = nc.declare_dram_parameter("out", [P, 4 * n_tiles], f32, isOutput=True)

    with TileContext(nc) as tc:
        with (
            tc.tile_pool(name="io", bufs=5) as io_pool,
            tc.tile_pool(name="mid", bufs=3) as mid_pool,
            tc.tile_pool(name="acc", bufs=1) as acc_pool,
        ):
            # col j: sum d; cols [n,2n): sum e^2; [2n,3n): relu(d); [3n,4n): relu(z)
            acc_d = acc_pool.tile([P, n_tiles], f32, tag="acc_d")
            acc_a = acc_pool.tile([P, 3 * n_tiles], f32, tag="acc_a")

            off = 0
            for j, fd in enumerate(TILE_WIDTHS):
                big_t = io_pool.tile([P, 5, fd], bf16, tag="big", name=f"big{j}")
                src = big[:, off : off + 5 * fd].rearrange("p (s f) -> p s f", s=5)
                nc.sync.dma_start(out=big_t[:, 0:2, :], in_=src[:, 0:2, :])
                nc.sync.dma_start(out=big_t[:, 2:5, :], in_=src[:, 2:5, :])
                off += 5 * fd

                lo = big_t[:, 0, :]
                up = big_t[:, 1, :]
                t_t = big_t[:, 2, :]
                p_t = big_t[:, 3, :]
                sg = big_t[:, 4, :]

                h = mid_pool.tile([P, fd], bf16, tag="h", name=f"h{j}")
                c = mid_pool.tile([P, fd], bf16, tag="c", name=f"c{j}")
                d = mid_pool.tile([P, fd], bf16, tag="d", name=f"d{j}")
                e = mid_pool.tile([P, fd], bf16, tag="e", name=f"e{j}")
                x = mid_pool.tile([P, fd], bf16, tag="x", name=f"x{j}")
                z = mid_pool.tile([P, fd], bf16, tag="z", name=f"z{j}")

                nc.vector.tensor_add(out=h, in0=lo, in1=up)
                nc.vector.tensor_scalar(
                    out=c, in0=h, scalar1=0.5, scalar2=None, op0=Alu.mult
                )
                nc.vector.tensor_sub(out=e, in0=c, in1=t_t)
                nc.vector.tensor_sub(out=x, in0=c, in1=p_t)
                nc.vector.tensor_mul(out=z, in0=sg, in1=x)
                # d = lo - c, fused width reduction (STT is 1x but carries acc);
                # placed last so ACT's inputs e/z are produced early
                nc.vector.scalar_tensor_tensor(
                    out=d, in0=lo, scalar=1.0, in1=c,
                    op0=Alu.mult, op1=Alu.subtract,
                    accum_out=acc_d[:, j : j + 1],
                )

                ja = mid_pool.tile([P, fd], bf16, tag="ja", name=f"ja{j}")
                nc.scalar.activation(
                    out=ja, in_=e, func=Act.Square,
                    accum_out=acc_a[:, j : j + 1],
                )
                nc.scalar.activation(
                    out=ja, in_=z, func=Act.Relu,
                    accum_out=acc_a[:, 2 * n_tiles + j : 2 * n_tiles + j + 1],
                )
                nc.scalar.activation(
                    out=ja, in_=d, func=Act.Relu,
                    accum_out=acc_a[:, n_tiles + j : n_tiles + j + 1],
                )

            nc.sync.dma_start(out=out[:, 0:n_tiles], in_=acc_d)
            nc.sync.dma_start(out=out[:, n_tiles : 4 * n_tiles], in_=acc_a)

    nc.compile()
    return nc


def _get_nc():
    if "nc" not in _NC_CACHE:
        _NC_CACHE["nc"] = _build()
    return _NC_CACHE["nc"]


def _shard(inputs):
    import ml_dtypes

    bf = ml_dtypes.bfloat16
    pred = np.asarray(inputs["pred"])
    targ = np.asarray(inputs["target"]).reshape(N)
    prev = np.asarray(inputs["prev_pci"]).reshape(N)
    pv = np.asarray(inputs["pv_values"]).reshape(N)

    lo = pred[:, 0].astype(bf)
    up = pred[:, 1].astype(bf)
    tb = targ.astype(bf)
    pb = prev.astype(bf)
    # sigma = +1 where pv==0 else -1: a bijective recode of the binary flag
    sg = np.where(pv == 0, np.float32(1.0), np.float32(-1.0)).astype(bf)

    in_maps = []
    for cix in range(N_CORES):
        s = slice(cix * NP_PER_CORE, (cix + 1) * NP_PER_CORE)
        streams = (
            lo[s].reshape(P, FPL),
            up[s].reshape(P, FPL),
            tb[s].reshape(P, FPL),
            pb[s].reshape(P, FPL),
            sg[s].reshape(P, FPL),
        )
        parts = []
        off = 0
        for fd in TILE_WIDTHS:
            for st in streams:
                parts.append(st[:, off : off + fd])
            off += fd
        big = np.concatenate(parts, axis=1)
        in_maps.append({"big": np.ascontiguousarray(big)})
    return in_maps


def _combine(core_outs, n_tiles=len(TILE_WIDTHS), n=N):
    allp = np.stack([np.asarray(o, dtype=np.float64) for o in core_outs])
    s = allp.reshape(len(core_outs), P, 4, n_tiles).sum(axis=(0, 1, 3))
    s_d, s_sq, s_rd, s_rz = s
    center_loss = s_sq / n
    width_loss = -2.0 * s_d / n
    valid_penalty = 2.0 * s_rd / n
    direction_penalty = s_rz
    total = (
        center_loss * 10.0
        + 0.1 * width_loss
        + 10.0 * valid_penalty
        + 0.5 * direction_penalty / n
    )
    return np.array(total, dtype=np.float32)


def _run(inputs, trace=False):
    from concourse.bass_utils import run_bass_kernel_spmd

    nc = _get_nc()
    in_maps = _shard(inputs)
    res = run_bass_kernel_spmd(
        nc, in_maps, core_ids=list(range(N_CORES)), trace=trace
    )
    core_outs = [res.results[c]["out"] for c in range(N_CORES)]
    return _combine(core_outs), res


def kernel(**inputs) -> np.ndarray:
    result, _ = _run(inputs, trace=False)
    return result


# revision 4
# speedup vs baseline: 1.0678x; 1.0678x over previous
"""Trainium2 Bass kernel for the interval-prediction custom loss.

total = 10*mean((t - c)^2) + 0.1*mean(up-lo) + 10*mean(relu(lo-up))
        + 0.5*sum(relu(sigma*(c-p)))/N,  c = (lo+up)/2, sigma = 1-2*pv.

Data parallel over N across 8 cores. Streams per core: lo, up, t, p,
sg (= 1-2*pv as bf16, exact +-1). Per tile of FD columns:

  DVE: h = lo + up                 (TT 2x)
       c = 0.5*h                   (TS 4x)
       d = lo - c (+acc sum d)     (STT 1x, fused width reduction)
         [note d = (lo-up)/2, so width = -2*sum(d), valid relu(lo-up)=2*relu(d)]
       e = c - t                   (TT 2x)
       x = c - p                   (TT 2x)
       z = sg * x                  (TT 2x)
  ACT: sum e^2   (Square acc)
       sum relu(d)  -> valid = 2*relu(d)
       sum relu(z)  -> direction

All reductions land in [P, n_tiles] accumulators; host does the tiny
final combine in float64.
"""

import sys

if "/opt/trn_rl_repo" not in sys.path:
    sys.path.insert(0, "/opt/trn_rl_repo")

import numpy as np

N = 8388608
N_CORES = 8
P = 128
NP_PER_CORE = N // N_CORES            # 1048576
FPL = NP_PER_CORE // P                # 8192
TILE_WIDTHS = (1024, 2048, 2304, 2304, 512)
assert sum(TILE_WIDTHS) == FPL

_NC_CACHE = {}


def _build():
    from concourse import bacc, mybir
    from concourse.tile import TileContext

    f32 = mybir.dt.float32
    bf16 = mybir.dt.bfloat16
    Alu = mybir.AluOpType
    Act = mybir.ActivationFunctionType

    n_tiles = len(TILE_WIDTHS)
    nc = bacc.Bacc(trn_type="TRN2")
    big = nc.declare_dram_parameter("big", [P, 5 * FPL], bf16, isOutput=False)
    out = nc.declare_dram_parameter("out", [P, 4 * n_tiles], f32, isOutput=True)

    with TileContext(nc) as tc:
        with (
            tc.tile_pool(name="io", bufs=5) as io_pool,
            tc.tile_pool(name="mid", bufs=3) as mid_pool,
            tc.tile_pool(name="acc", bufs=1) as acc_pool,
        ):
            # col j: sum d; cols [n,2n): sum e^2; [2n,3n): relu(d); [3n,4n): relu(z)
            acc_all = acc_pool.tile([P, 4 * n_tiles], f32, tag="acc_all")
            acc_d = acc_all[:, 0:n_tiles]
            acc_a = acc_all[:, n_tiles : 4 * n_tiles]

            off = 0
            for j, fd in enumerate(TILE_WIDTHS):
                big_t = io_pool.tile([P, 5, fd], bf16, tag="big", name=f"big{j}")
                src = big[:, off : off + 5 * fd].rearrange("p (s f) -> p s f", s=5)
                nc.sync.dma_start(out=big_t[:, 0:2, :], in_=src[:, 0:2, :])
                nc.sync.dma_start(out=big_t[:, 2:5, :], in_=src[:, 2:5, :])
                off += 5 * fd

                lo = big_t[:, 0, :]
                up = big_t[:, 1, :]
                t_t = big_t[:, 2, :]
                p_t = big_t[:, 3, :]
                sg = big_t[:, 4, :]

                h = mid_pool.tile([P, fd], bf16, tag="h", name=f"h{j}")
                d = mid_pool.tile([P, fd], bf16, tag="d", name=f"d{j}")
                e = mid_pool.tile([P, fd], bf16, tag="e", name=f"e{j}")
                x = mid_pool.tile([P, fd], bf16, tag="x", name=f"x{j}")
                z = mid_pool.tile([P, fd], bf16, tag="z", name=f"z{j}")

                # H = lo+up = 2c; streams t,p arrive pre-doubled (exact bf16
                # exponent shift), so e/x need no 0.5*H op:
                #   E = 2t - H = 2(t-c), X = 2p - H = 2(p-c), z = sgm*X
                nc.vector.tensor_add(out=h, in0=lo, in1=up)
                # D = lo - up with fused width acc; only needs the first DMA
                nc.vector.scalar_tensor_tensor(
                    out=d, in0=lo, scalar=1.0, in1=up,
                    op0=Alu.mult, op1=Alu.subtract,
                    accum_out=acc_d[:, j : j + 1],
                )
                nc.vector.tensor_sub(out=e, in0=t_t, in1=h)
                nc.vector.tensor_sub(out=x, in0=p_t, in1=h)
                nc.vector.tensor_mul(out=z, in0=sg, in1=x)

                ja = mid_pool.tile([P, fd], bf16, tag="ja", name=f"ja{j}")
                nc.scalar.activation(
                    out=ja, in_=d, func=Act.Relu,
                    accum_out=acc_a[:, n_tiles + j : n_tiles + j + 1],
                )
                nc.scalar.activation(
                    out=ja, in_=e, func=Act.Square,
                    accum_out=acc_a[:, j : j + 1],
                )
                nc.scalar.activation(
                    out=ja, in_=z, func=Act.Relu,
                    accum_out=acc_a[:, 2 * n_tiles + j : 2 * n_tiles + j + 1],
                )

            nc.sync.dma_start(out=out[:, :], in_=acc_all)

    nc.compile()
    return nc


def _get_nc():
    if "nc" not in _NC_CACHE:
        _NC_CACHE["nc"] = _build()
    return _NC_CACHE["nc"]


def _shard(inputs):
    import ml_dtypes

    bf = ml_dtypes.bfloat16
    pred = np.asarray(inputs["pred"])
    targ = np.asarray(inputs["target"]).reshape(N)
    prev = np.asarray(inputs["prev_pci"]).reshape(N)
    pv = np.asarray(inputs["pv_values"]).reshape(N)

    lo = pred[:, 0].astype(bf)
    up = pred[:, 1].astype(bf)
    tb = (2.0 * targ).astype(bf)
    pb = (2.0 * prev).astype(bf)
    # sigma = +1 where pv==0 else -1: a bijective recode of the binary flag
    sg = np.where(pv == 0, np.float32(-1.0), np.float32(1.0)).astype(bf)

    in_maps = []
    for cix in range(N_CORES):
        s = slice(cix * NP_PER_CORE, (cix + 1) * NP_PER_CORE)
        streams = (
            lo[s].reshape(P, FPL),
            up[s].reshape(P, FPL),
            tb[s].reshape(P, FPL),
            pb[s].reshape(P, FPL),
            sg[s].reshape(P, FPL),
        )
        parts = []
        off = 0
        for fd in TILE_WIDTHS:
            for st in streams:
                parts.append(st[:, off : off + fd])
            off += fd
        big = np.concatenate(parts, axis=1)
        in_maps.append({"big": np.ascontiguousarray(big)})
    return in_maps


def _combine(core_outs, n_tiles=len(TILE_WIDTHS), n=N):
    allp = np.stack([np.asarray(o, dtype=np.float64) for o in core_outs])
    s = allp.reshape(len(core_outs), P, 4, n_tiles).sum(axis=(0, 1, 3))
    s_d, s_sq, s_rd, s_rz = s
    center_loss = 0.25 * s_sq / n
    width_loss = -s_d / n
    valid_penalty = s_rd / n
    direction_penalty = 0.5 * s_rz
    total = (
        center_loss * 10.0
        + 0.1 * width_loss
        + 10.0 * valid_penalty
        + 0.5 * direction_penalty / n
    )
    return np.array(total, dtype=np.float32)


def _run(inputs, trace=False):
    from concourse.bass_utils import run_bass_kernel_spmd

    nc = _get_nc()
    in_maps = _shard(inputs)
    res = run_bass_kernel_spmd(
        nc, in_maps, core_ids=list(range(N_CORES)), trace=trace
    )
    core_outs = [res.results[c]["out"] for c in range(N_CORES)]
    return _combine(core_outs), res


def kernel(**inputs) -> np.ndarray:
    result, _ = _run(inputs, trace=False)
    return result


# revision 5
# speedup vs baseline: 1.0948x; 1.0252x over previous
"""Trainium2 Bass kernel: pv-sorted layout, 4 streams, c-free math.

total = 10*mean((t-c)^2) + 0.1*mean(up-lo) + 10*mean(relu(lo-up))
        + 0.5*sum(where(pv==0, relu(c-p), relu(p-c)))/N,  c = (lo+up)/2.

All loss terms are permutation-invariant sums, so the host sorts each
core's elements by pv (pure layout) in column-major order: columns
< C_STAR are all pv=0, columns > C_STAR all pv=1, and the single
boundary column C_STAR is handled with a per-partition +-1 scale
vector fed to ACT's scale-AP. This removes the sigma stream (DMA
10 -> 8 B/elem) and the z=sg*x DVE op.

Host pre-doubles t,p (exact bf16 exponent shift). Per tile:
  DVE: H = lo+up (TT 2x), E = 2t-H (TT), X = 2p-H (TT),
       D = lo-up (STT 1x, fused width acc)
  ACT: sum E^2 (Square), sum relu(D), sum relu(-X)/relu(+X)/relu(sc*X)
       per pv-class column range.
Host: center = 0.25*sum(E^2)/N, width = -sum(D)/N,
      valid = sum(relu(D))/N, direction = 0.5*sum(relu ranges).
"""

import sys

if "/opt/trn_rl_repo" not in sys.path:
    sys.path.insert(0, "/opt/trn_rl_repo")

import numpy as np

N = 8388608
N_CORES = 8
P = 128
NP_PER_CORE = N // N_CORES
FPL = NP_PER_CORE // P                # 8192
TILE_WIDTHS = (1024, 2048, 2304, 2304, 512)
assert sum(TILE_WIDTHS) == FPL

_NC_CACHE = {}


def _build(c_star):
    from concourse import bacc, mybir
    from concourse.tile import TileContext

    f32 = mybir.dt.float32
    bf16 = mybir.dt.bfloat16
    Alu = mybir.AluOpType
    Act = mybir.ActivationFunctionType

    n_tiles = len(TILE_WIDTHS)
    nrx = n_tiles + 2                 # max relu-X accumulator slots
    nc = bacc.Bacc(trn_type="TRN2")
    big = nc.declare_dram_parameter("big", [P, 4 * FPL], bf16, isOutput=False)
    scp = nc.declare_dram_parameter("scp", [P, 1], f32, isOutput=False)
    out = nc.declare_dram_parameter(
        "out", [P, 3 * n_tiles + nrx], f32, isOutput=True
    )

    with TileContext(nc) as tc:
        with (
            tc.tile_pool(name="io", bufs=5) as io_pool,
            tc.tile_pool(name="mid", bufs=3) as mid_pool,
            tc.tile_pool(name="acc", bufs=1) as acc_pool,
        ):
            # [sum D | sum E^2 | relu D | relu X slots]
            acc_all = acc_pool.tile([P, 3 * n_tiles + nrx], f32, tag="acc")
            nc.vector.memset(acc_all[:, :], 0.0)
            sct = acc_pool.tile([P, 1], f32, tag="sct")
            nc.sync.dma_start(out=sct, in_=scp[:, :])

            rx_slot = [0]

            def relu_x_ranges(off, fd):
                """(lo_col, hi_col, scale) pieces of [off, off+fd)."""
                pieces = []
                a0, a1 = off, min(off + fd, c_star)
                if a1 > a0:
                    pieces.append((a0, a1, -1.0))        # pv=0: relu(-X)
                b0, b1 = max(off, c_star), min(off + fd, c_star + 1)
                if b1 > b0:
                    pieces.append((b0, b1, None))        # boundary col: sc AP
                d0, d1 = max(off, c_star + 1), off + fd
                if d1 > d0:
                    pieces.append((d0, d1, 1.0))         # pv=1: relu(+X)
                return pieces

            off = 0
            for j, fd in enumerate(TILE_WIDTHS):
                big_t = io_pool.tile([P, 4, fd], bf16, tag="big", name=f"big{j}")
                src = big[:, off * 4 : (off + fd) * 4].rearrange(
                    "p (s f) -> p s f", s=4
                )
                nc.sync.dma_start(out=big_t[:, 0:2, :], in_=src[:, 0:2, :])
                nc.sync.dma_start(out=big_t[:, 2:4, :], in_=src[:, 2:4, :])

                lo = big_t[:, 0, :]
                up = big_t[:, 1, :]
                t_t = big_t[:, 2, :]
                p_t = big_t[:, 3, :]

                h = mid_pool.tile([P, fd], bf16, tag="h", name=f"h{j}")
                d = mid_pool.tile([P, fd], bf16, tag="d", name=f"d{j}")
                e = mid_pool.tile([P, fd], bf16, tag="e", name=f"e{j}")
                x = mid_pool.tile([P, fd], bf16, tag="x", name=f"x{j}")

                nc.vector.tensor_add(out=h, in0=lo, in1=up)
                nc.vector.scalar_tensor_tensor(
                    out=d, in0=lo, scalar=1.0, in1=up,
                    op0=Alu.mult, op1=Alu.subtract,
                    accum_out=acc_all[:, j : j + 1],
                )
                nc.vector.tensor_sub(out=e, in0=t_t, in1=h)
                nc.vector.tensor_sub(out=x, in0=p_t, in1=h)

                ja = mid_pool.tile([P, fd], bf16, tag="ja", name=f"ja{j}")
                nc.scalar.activation(
                    out=ja, in_=d, func=Act.Relu,
                    accum_out=acc_all[:, 2 * n_tiles + j : 2 * n_tiles + j + 1],
                )
                nc.scalar.activation(
                    out=ja, in_=e, func=Act.Square,
                    accum_out=acc_all[:, n_tiles + j : n_tiles + j + 1],
                )
                for (r0, r1, scl) in relu_x_ranges(off, fd):
                    k = 3 * n_tiles + rx_slot[0]
                    rx_slot[0] += 1
                    xs = x[:, r0 - off : r1 - off]
                    js = ja[:, r0 - off : r1 - off]
                    if scl is None:
                        nc.scalar.activation(
                            out=js, in_=xs, func=Act.Relu,
                            scale=sct[:, 0:1],
                            accum_out=acc_all[:, k : k + 1],
                        )
                    else:
                        nc.scalar.activation(
                            out=js, in_=xs, func=Act.Relu, scale=scl,
                            accum_out=acc_all[:, k : k + 1],
                        )
                off += fd

            nc.sync.dma_start(out=out[:, :], in_=acc_all)

    nc.compile()
    return nc


def _get_nc(c_star):
    if c_star not in _NC_CACHE:
        _NC_CACHE[c_star] = _build(c_star)
    return _NC_CACHE[c_star]


def _shard(inputs):
    import ml_dtypes

    bf = ml_dtypes.bfloat16
    pred = np.asarray(inputs["pred"])
    lo_a = pred[:, 0].astype(np.float32)
    up_a = pred[:, 1].astype(np.float32)
    t_a = (2.0 * np.asarray(inputs["target"]).reshape(N)).astype(np.float32)
    p_a = (2.0 * np.asarray(inputs["prev_pci"]).reshape(N)).astype(np.float32)
    pv = np.asarray(inputs["pv_values"]).reshape(N)

    # global pv-sort (stable) and balanced per-core class counts
    c0 = np.flatnonzero(pv == 0)
    c1 = np.flatnonzero(pv != 0)
    B = len(c0)
    b_lo = B // N_CORES
    counts0 = [b_lo + (1 if i < B % N_CORES else 0) for i in range(N_CORES)]
    c_star = b_lo // P

    ofs0 = np.concatenate([[0], np.cumsum(counts0)])
    ofs1 = np.concatenate([[0], np.cumsum([NP_PER_CORE - c for c in counts0])])

    in_maps = []
    sc_list = []
    for i in range(N_CORES):
        idx = np.concatenate([
            c0[ofs0[i] : ofs0[i + 1]], c1[ofs1[i] : ofs1[i + 1]]
        ])
        assert len(idx) == NP_PER_CORE
        r_c = counts0[i] - c_star * P          # in [0, 128]
        sc = np.where(np.arange(P) < r_c, np.float32(-1.0), np.float32(1.0))
        sc_list.append(np.ascontiguousarray(sc.reshape(P, 1).astype(np.float32)))

        # column-major fill: element m -> (partition m % P, column m // P)
        def cm(a):
            return np.ascontiguousarray(a[idx].astype(bf).reshape(FPL, P).T)

        st = (cm(lo_a), cm(up_a), cm(t_a), cm(p_a))
        parts = []
        off = 0
        for fd in TILE_WIDTHS:
            for s_ in st:
                parts.append(s_[:, off : off + fd])
            off += fd
        big = np.concatenate(parts, axis=1)
        in_maps.append({"big": np.ascontiguousarray(big), "scp": sc_list[i]})
    return in_maps, c_star


def _combine(core_outs, n_tiles=len(TILE_WIDTHS), n=N):
    nrx = n_tiles + 2
    allp = np.stack([np.asarray(o, dtype=np.float64) for o in core_outs])
    s_d = allp[:, :, 0:n_tiles].sum()
    s_sq = allp[:, :, n_tiles : 2 * n_tiles].sum()
    s_rd = allp[:, :, 2 * n_tiles : 3 * n_tiles].sum()
    s_rx = allp[:, :, 3 * n_tiles : 3 * n_tiles + nrx].sum()
    center_loss = 0.25 * s_sq / n
    width_loss = -s_d / n
    valid_penalty = s_rd / n
    direction_penalty = 0.5 * s_rx
    total = (
        center_loss * 10.0
        + 0.1 * width_loss
        + 10.0 * valid_penalty
        + 0.5 * direction_penalty / n
    )
    return np.array(total, dtype=np.float32)


def _run(inputs, trace=False):
    from concourse.bass_utils import run_bass_kernel_spmd

    in_maps, c_star = _shard(inputs)
    nc = _get_nc(c_star)
    res = run_bass_kernel_spmd(
        nc, in_maps, core_ids=list(range(N_CORES)), trace=trace
    )
    core_outs = [res.results[c]["out"] for c in range(N_CORES)]
    return _combine(core_outs), res


def kernel(**inputs) -> np.ndarray:
    result, _ = _run(inputs, trace=False)
    return result


# revision 6
# speedup vs baseline: 1.1020x; 1.0066x over previous
"""Trainium2 Bass kernel: pv-sorted layout, 4 streams, c-free math.

total = 10*mean((t-c)^2) + 0.1*mean(up-lo) + 10*mean(relu(lo-up))
        + 0.5*sum(where(pv==0, relu(c-p), relu(p-c)))/N,  c = (lo+up)/2.

All loss terms are permutation-invariant sums, so the host sorts each
core's elements by pv (pure layout) in column-major order: columns
< C_STAR are all pv=0, columns > C_STAR all pv=1, and the single
boundary column C_STAR is handled with a per-partition +-1 scale
vector fed to ACT's scale-AP. This removes the sigma stream (DMA
10 -> 8 B/elem) and the z=sg*x DVE op.

Host pre-doubles t,p (exact bf16 exponent shift). Per tile:
  DVE: H = lo+up (TT 2x), E = 2t-H (TT), X = 2p-H (TT),
       D = lo-up (STT 1x, fused width acc)
  ACT: sum E^2 (Square), sum relu(D), sum relu(-X)/relu(+X)/relu(sc*X)
       per pv-class column range.
Host: center = 0.25*sum(E^2)/N, width = -sum(D)/N,
      valid = sum(relu(D))/N, direction = 0.5*sum(relu ranges).
"""

import sys

if "/opt/trn_rl_repo" not in sys.path:
    sys.path.insert(0, "/opt/trn_rl_repo")

import numpy as np

N = 8388608
N_CORES = 8
P = 128
NP_PER_CORE = N // N_CORES
FPL = NP_PER_CORE // P                # 8192
TILE_WIDTHS = (1024, 2304, 2560, 2304)
assert sum(TILE_WIDTHS) == FPL

_NC_CACHE = {}


def _build(c_star):
    from concourse import bacc, mybir
    from concourse.tile import TileContext

    f32 = mybir.dt.float32
    bf16 = mybir.dt.bfloat16
    Alu = mybir.AluOpType
    Act = mybir.ActivationFunctionType

    n_tiles = len(TILE_WIDTHS)
    nrx = n_tiles + 2                 # max relu-X accumulator slots
    nc = bacc.Bacc(trn_type="TRN2")
    big = nc.declare_dram_parameter("big", [P, 4 * FPL], bf16, isOutput=False)
    scp = nc.declare_dram_parameter("scp", [P, 1], f32, isOutput=False)
    out = nc.declare_dram_parameter(
        "out", [P, 3 * n_tiles + nrx], f32, isOutput=True
    )

    with TileContext(nc) as tc:
        with (
            tc.tile_pool(name="io", bufs=5) as io_pool,
            tc.tile_pool(name="mid", bufs=3) as mid_pool,
            tc.tile_pool(name="acc", bufs=1) as acc_pool,
        ):
            # [sum D | sum E^2 | relu D | relu X slots]
            acc_all = acc_pool.tile([P, 3 * n_tiles + nrx], f32, tag="acc")
            nc.vector.memset(acc_all[:, :], 0.0)
            sct = acc_pool.tile([P, 1], f32, tag="sct")
            nc.sync.dma_start(out=sct, in_=scp[:, :])

            rx_slot = [0]

            def relu_x_ranges(off, fd):
                """(lo_col, hi_col, scale) pieces of [off, off+fd)."""
                pieces = []
                a0, a1 = off, min(off + fd, c_star)
                if a1 > a0:
                    pieces.append((a0, a1, -1.0))        # pv=0: relu(-X)
                b0, b1 = max(off, c_star), min(off + fd, c_star + 1)
                if b1 > b0:
                    pieces.append((b0, b1, None))        # boundary col: sc AP
                d0, d1 = max(off, c_star + 1), off + fd
                if d1 > d0:
                    pieces.append((d0, d1, 1.0))         # pv=1: relu(+X)
                return pieces

            off = 0
            for j, fd in enumerate(TILE_WIDTHS):
                big_t = io_pool.tile([P, 4, fd], bf16, tag="big", name=f"big{j}")
                src = big[:, off * 4 : (off + fd) * 4].rearrange(
                    "p (s f) -> p s f", s=4
                )
                nc.sync.dma_start(out=big_t[:, 0:2, :], in_=src[:, 0:2, :])
                nc.sync.dma_start(out=big_t[:, 2:4, :], in_=src[:, 2:4, :])

                lo = big_t[:, 0, :]
                up = big_t[:, 1, :]
                t_t = big_t[:, 2, :]
                p_t = big_t[:, 3, :]

                h = mid_pool.tile([P, fd], bf16, tag="h", name=f"h{j}")
                d = mid_pool.tile([P, fd], bf16, tag="d", name=f"d{j}")
                e = mid_pool.tile([P, fd], bf16, tag="e", name=f"e{j}")
                x = mid_pool.tile([P, fd], bf16, tag="x", name=f"x{j}")

                nc.vector.tensor_add(out=h, in0=lo, in1=up)
                nc.vector.scalar_tensor_tensor(
                    out=d, in0=lo, scalar=1.0, in1=up,
                    op0=Alu.mult, op1=Alu.subtract,
                    accum_out=acc_all[:, j : j + 1],
                )
                nc.vector.tensor_sub(out=e, in0=t_t, in1=h)
                nc.vector.tensor_sub(out=x, in0=p_t, in1=h)

                ja = mid_pool.tile([P, fd], bf16, tag="ja", name=f"ja{j}")
                nc.scalar.activation(
                    out=ja, in_=d, func=Act.Relu,
                    accum_out=acc_all[:, 2 * n_tiles + j : 2 * n_tiles + j + 1],
                )
                nc.scalar.activation(
                    out=ja, in_=e, func=Act.Square,
                    accum_out=acc_all[:, n_tiles + j : n_tiles + j + 1],
                )
                for (r0, r1, scl) in relu_x_ranges(off, fd):
                    k = 3 * n_tiles + rx_slot[0]
                    rx_slot[0] += 1
                    xs = x[:, r0 - off : r1 - off]
                    js = ja[:, r0 - off : r1 - off]
                    if scl is None:
                        nc.scalar.activation(
                            out=js, in_=xs, func=Act.Relu,
                            scale=sct[:, 0:1],
                            accum_out=acc_all[:, k : k + 1],
                        )
                    else:
                        nc.scalar.activation(
                            out=js, in_=xs, func=Act.Relu, scale=scl,
                            accum_out=acc_all[:, k : k + 1],
                        )
                off += fd

            nc.sync.dma_start(out=out[:, :], in_=acc_all)

    nc.compile()
    return nc


def _get_nc(c_star):
    if c_star not in _NC_CACHE:
        _NC_CACHE[c_star] = _build(c_star)
    return _NC_CACHE[c_star]


def _shard(inputs):
    import ml_dtypes

    bf = ml_dtypes.bfloat16
    pred = np.asarray(inputs["pred"])
    lo_a = pred[:, 0].astype(np.float32)
    up_a = pred[:, 1].astype(np.float32)
    t_a = (2.0 * np.asarray(inputs["target"]).reshape(N)).astype(np.float32)
    p_a = (2.0 * np.asarray(inputs["prev_pci"]).reshape(N)).astype(np.float32)
    pv = np.asarray(inputs["pv_values"]).reshape(N)

    # global pv-sort (stable) and balanced per-core class counts
    c0 = np.flatnonzero(pv == 0)
    c1 = np.flatnonzero(pv != 0)
    B = len(c0)
    b_lo = B // N_CORES
    counts0 = [b_lo + (1 if i < B % N_CORES else 0) for i in range(N_CORES)]
    c_star = b_lo // P

    ofs0 = np.concatenate([[0], np.cumsum(counts0)])
    ofs1 = np.concatenate([[0], np.cumsum([NP_PER_CORE - c for c in counts0])])

    in_maps = []
    sc_list = []
    for i in range(N_CORES):
        idx = np.concatenate([
            c0[ofs0[i] : ofs0[i + 1]], c1[ofs1[i] : ofs1[i + 1]]
        ])
        assert len(idx) == NP_PER_CORE
        r_c = counts0[i] - c_star * P          # in [0, 128]
        sc = np.where(np.arange(P) < r_c, np.float32(-1.0), np.float32(1.0))
        sc_list.append(np.ascontiguousarray(sc.reshape(P, 1).astype(np.float32)))

        # column-major fill: element m -> (partition m % P, column m // P)
        def cm(a):
            return np.ascontiguousarray(a[idx].astype(bf).reshape(FPL, P).T)

        st = (cm(lo_a), cm(up_a), cm(t_a), cm(p_a))
        parts = []
        off = 0
        for fd in TILE_WIDTHS:
            for s_ in st:
                parts.append(s_[:, off : off + fd])
            off += fd
        big = np.concatenate(parts, axis=1)
        in_maps.append({"big": np.ascontiguousarray(big), "scp": sc_list[i]})
    return in_maps, c_star


def _combine(core_outs, n_tiles=len(TILE_WIDTHS), n=N):
    nrx = n_tiles + 2
    allp = np.stack([np.asarray(o, dtype=np.float64) for o in core_outs])
    s_d = allp[:, :, 0:n_tiles].sum()
    s_sq = allp[:, :, n_tiles : 2 * n_tiles].sum()
    s_rd = allp[:, :, 2 * n_tiles : 3 * n_tiles].sum()
    s_rx = allp[:, :, 3 * n_tiles : 3 * n_tiles + nrx].sum()
    center_loss = 0.25 * s_sq / n
    width_loss = -s_d / n
    valid_penalty = s_rd / n
    direction_penalty = 0.5 * s_rx
    total = (
        center_loss * 10.0
        + 0.1 * width_loss
        + 10.0 * valid_penalty
        + 0.5 * direction_penalty / n
    )
    return np.array(total, dtype=np.float32)


def _run(inputs, trace=False):
    from concourse.bass_utils import run_bass_kernel_spmd

    in_maps, c_star = _shard(inputs)
    nc = _get_nc(c_star)
    res = run_bass_kernel_spmd(
        nc, in_maps, core_ids=list(range(N_CORES)), trace=trace
    )
    core_outs = [res.results[c]["out"] for c in range(N_CORES)]
    return _combine(core_outs), res


def kernel(**inputs) -> np.ndarray:
    result, _ = _run(inputs, trace=False)
    return result
